# revision 12
# baseline (speedup 1.0000x reference)
"""Trainium2 Bass kernel for BertLinearSelfAttention (linear attention).

Reference computation (per batch b, head h):
    q,k,v = X @ W{q,k,v} + b{q,k,v}            # [S, D] -> heads of 64
    qf, kf = elu(q)+1, elu(k)+1                # = min(exp(x),1) + max(x,0)
    kv[d,e]  = sum_s kf[s,d] v[s,e]            # [64, 64]
    ksum[d]  = sum_s kf[s,d]
    out[s,e] = (sum_d qf[s,d] kv[d,e]) / (sum_d qf[s,d] ksum[d])

Sharding: 8 cores = (4 batches) x (2 head-groups of 8 heads / 512 proj cols).
X is fed pre-transposed ([D, S]) and everything the PE touches is bf16
(inputs quantized host-side): bf16 streams at 1 cycle/row at any free size
(fp32r drops to 4 cycles/row under N=256) and halves LDWEIGHTS + DMA.

Pass A (per 512-token chunk): k/v projections; k feature map via in-place
PSUM bias add (DVE) -> Exp + Relu (ACT) -> fused min/add combine (DVE stt,
16-bit 2x mode); v copied to SBUF raw (ACT) with a const-1 column per head
riding along; kv/ksum accumulated with 2-heads-per-matmul (N=132).
bv is folded in once at the end as a rank-1 ksum x bv correction.

Pass B: q^T projection (bq applied for free via ACT per-partition bias),
block-diagonal num matmuls whose kv blocks carry the ksum columns so the
denominator rides in the same matmul (N=130), then one reciprocal [128,2]
+ one broadcast multiply per (ct, sub) on DVE. Output stored bf16 and
upcast host-side. X^T (bf16, 64 KiB/partition) stays resident in SBUF so
pass B never re-reads HBM.
"""

import os
import sys

import numpy as np

_REPO = "/opt/trn_rl_repo"
if os.path.isdir(_REPO) and _REPO not in sys.path:
    sys.path.insert(0, _REPO)

B, S, D, H, HD = 4, 4096, 1024, 16, 64
NCORES = 8
CG = 512            # projection columns per core (8 heads)
NH = CG // HD       # 8 heads per core
HE = HD + 2         # head cols in V' ([v | 1 | 1-pad])
NCT = CG // 128     # 4 column tiles (head pairs)
KB = 130            # kv-block cols: 2x64 num + 2 ksum cols
CHUNK = 512         # tokens per chunk
NSUB = CHUNK // 128     # 4 token sub-tiles per chunk
NCHUNK = S // CHUNK     # 8 chunks
NKT = D // 128          # 8 contraction tiles
P = 128

_CACHED_NC = None


def _build():
    import concourse.tile as tile
    from concourse import bacc, mybir
    from contextlib import ExitStack

    F32 = mybir.dt.float32
    BF16 = mybir.dt.bfloat16
    Alu = mybir.AluOpType
    Act = mybir.ActivationFunctionType

    nc = bacc.Bacc("TRN2", target_bir_lowering=False, debug=False,
                   num_devices=NCORES)

    xt_d = nc.dram_tensor("xt", [D, S], BF16, kind="ExternalInput").ap()
    w_d = {
        "q": nc.dram_tensor("wq", [D, CG], BF16, kind="ExternalInput").ap(),
        "k": nc.dram_tensor("wk", [D, CG], BF16, kind="ExternalInput").ap(),
        "v": nc.dram_tensor("wv", [D, CG], BF16, kind="ExternalInput").ap(),
    }
    bq_d = nc.dram_tensor("bq", [CG], F32, kind="ExternalInput").ap()
    bk_d = nc.dram_tensor("bk", [1, CG], F32, kind="ExternalInput").ap()
    bv_d = nc.dram_tensor("bv", [1, CG], F32, kind="ExternalInput").ap()
    out_d = nc.dram_tensor("out", [S, CG], BF16, kind="ExternalOutput").ap()

    with tile.TileContext(nc) as tc:
        with ExitStack() as ctx:
            const = ctx.enter_context(tc.tile_pool(name="const", bufs=1))
            wpool = ctx.enter_context(tc.tile_pool(name="wpool", bufs=1))
            # all 64 X^T tiles stay resident (1 KiB/partition each)
            xtpool = ctx.enter_context(
                tc.tile_pool(name="xtpool", bufs=NCHUNK * NKT))

            xt_t = {}

            def load_xt(ci):
                tok0 = ci * CHUNK
                xts = []
                for kt in range(NKT):
                    t = xtpool.tile([P, CHUNK], BF16, tag="xt", name="xt")
                    nc.sync.dma_start(
                        t[:], xt_d[kt * P:(kt + 1) * P, tok0:tok0 + CHUNK])
                    xts.append(t)
                xt_t[ci] = xts
                return xts

            # queue the first chunk's X^T ahead of all setup DMAs
            load_xt(0)

            # ---- constants / weights (one-time) ----
            # q bias per-partition: bq_sb[:, ct] = bq[ct*128:(ct+1)*128]
            bq_sb = const.tile([P, NCT], F32, tag="bqsb")
            nc.sync.dma_start(bq_sb[:], bq_d.rearrange("(c p) -> p c", p=P))
            nbq_sb = const.tile([P, NCT], F32, tag="nbqsb")
            nc.vector.tensor_scalar(nbq_sb[:], bq_sb[:], -1.0, None, Alu.mult)

            # k / v biases replicated to all partitions
            bk32 = const.tile([1, CG], F32, tag="bk32")
            nc.sync.dma_start(bk32[:], bk_d[:])
            bk_rep = const.tile([P, CG], F32, tag="bkrep")
            nc.gpsimd.partition_broadcast(bk_rep[:], bk32[:])
            bv32 = const.tile([1, CG], F32, tag="bv32")
            nc.sync.dma_start(bv32[:], bv_d[:])
            bv_rep = const.tile([P, CG], F32, tag="bvrep")
            nc.gpsimd.partition_broadcast(bv_rep[:], bv32[:])

            # weights bf16 straight from DRAM, in the order pass A consumes
            # them (all wk, then wv, then wq lazily), split across the sync
            # and gpsimd queues so the first chunk never stalls on them
            w_r = {}
            for nm in ("k", "v", "q"):
                w_r[nm] = wpool.tile([P, NKT * CG], BF16, tag=f"w{nm}r",
                                     name=f"w{nm}r")
            for nm in ("k", "v"):
                for kt in range(NKT):
                    q = nc.sync if kt % 2 == 0 else nc.gpsimd
                    q.dma_start(w_r[nm][:, kt * CG:(kt + 1) * CG],
                                w_d[nm][kt * P:(kt + 1) * P, :])
            for kt in range(NKT):
                nc.gpsimd.dma_start(w_r["q"][:, kt * CG:(kt + 1) * CG],
                                    w_d["q"][kt * P:(kt + 1) * P, :])

            # kv + ksum accumulator: per ct a [128, 132] block holding the
            # 2x2 head-pair cross products; only the diagonal blocks (and
            # their ksum columns at +64 / +66+64) are ever read.
            kv_sb = wpool.tile([P, NCT * 2 * HE], F32, tag="kvsb")
            nc.vector.memset(kv_sb[:], 0.0)
            # block-diag kv per c-tile (bf16): rows 0:64 head 2ct -> cols
            # 0:64 + ksum col 128; rows 64:128 head 2ct+1 -> cols 64:128 +
            # ksum col 129. Lets the num matmul carry the denominator.
            kvblocks = [wpool.tile([P, KB], BF16, tag=f"kvb{i}",
                                   name=f"kvb{i}") for i in range(NCT)]

            kfpool = ctx.enter_context(tc.tile_pool(name="kfpool", bufs=9))
            vppool = ctx.enter_context(tc.tile_pool(name="vppool", bufs=9))
            qftpool = ctx.enter_context(tc.tile_pool(name="qftpool", bufs=9))
            tmp = ctx.enter_context(tc.tile_pool(name="tmp", bufs=6))
            outpool = ctx.enter_context(tc.tile_pool(name="outp", bufs=6))
            rcpool = ctx.enter_context(tc.tile_pool(name="rcp", bufs=16))
            pps = ctx.enter_context(
                tc.tile_pool(name="pps", bufs=4, space="PSUM"))
            sps = ctx.enter_context(
                tc.tile_pool(name="sps", bufs=4, space="PSUM"))

            kf_c = {}   # chunk -> list of kf tiles (per sub)
            vp_c = {}
            qft_c = {}  # chunk -> list of q_feat^T tiles (per ctile)

            def a_chunk(ci):
                """Pass A for chunk ci: k/v projections + feature maps."""
                xts = xt_t.get(ci) or load_xt(ci)
                kfs, vps = [], []
                for nm in ("k", "v"):
                    for sub in range(NSUB):
                        ps = pps.tile([P, CG], F32, tag="pps", name="pps")
                        for kt in range(NKT):
                            nc.tensor.matmul(
                                ps[:],
                                xts[kt][:, sub * P:(sub + 1) * P],
                                w_r[nm][:, kt * CG:(kt + 1) * CG],
                                start=(kt == 0), stop=(kt == NKT - 1))
                        if nm == "k":
                            # biased k in place, then
                            # kf = min(exp(kb),1) + max(kb,0)
                            nc.vector.tensor_tensor(
                                ps[:], ps[:], bk_rep[:], Alu.add)
                            e = tmp.tile([P, CG], BF16, tag="te", name="t_e")
                            nc.scalar.activation(e[:], ps[:], Act.Exp)
                            r = tmp.tile([P, CG], BF16, tag="tr", name="t_r")
                            nc.scalar.activation(r[:], ps[:], Act.Relu)
                            kf = kfpool.tile([P, CG], BF16, tag="kf",
                                             name="kf")
                            nc.vector.scalar_tensor_tensor(
                                kf[:], e[:], 1.0, r[:], Alu.min, Alu.add)
                            kfs.append(kf)
                        else:
                            # V' = [v | 1 | 1] per head (bv folded in later
                            # as a rank-1 ksum x bv correction)
                            vp = vppool.tile([P, NH * HE], BF16, tag="vp",
                                             name="vp")
                            nc.gpsimd.memset(
                                vp[:].rearrange(
                                    "p (h e) -> p h e", e=HE)[:, :, HD:], 1.0)
                            nc.vector.tensor_copy(
                                vp[:].rearrange(
                                    "p (h e) -> p h e", e=HE)[:, :, :HD],
                                ps[:].rearrange("p (h e) -> p h e", e=HD))
                            vps.append(vp)
                kf_c[ci] = kfs
                vp_c[ci] = vps

            def a_kv(ci):
                """kv/ksum accumulation for chunk ci (head pair per matmul)."""
                kfs, vps = kf_c.pop(ci), vp_c.pop(ci)
                for ct in range(NCT):
                    kvt = sps.tile([P, 2 * HE], F32, tag="sps", name="kvt")
                    for sub in range(NSUB):
                        nc.tensor.matmul(
                            kvt[:],
                            kfs[sub][:, ct * P:(ct + 1) * P],
                            vps[sub][:, ct * 2 * HE:(ct + 1) * 2 * HE],
                            start=(sub == 0), stop=(sub == NSUB - 1))
                    acc = kv_sb[:, ct * 2 * HE:(ct + 1) * 2 * HE]
                    nc.vector.tensor_tensor(acc, acc, kvt[:], Alu.add)

            def build_kvblocks():
                """bv fold-in + block-diagonal bf16 kv blocks with ksum.

                kvb copies go to gpsimd (SBUF-only) so they overlap the DVE
                chain; the kvb memsets were already emitted earlier.
                """
                for ct in range(NCT):
                    base = ct * 2 * HE
                    # combined ksum column: rows 0:64 head 2ct, 64:128 2ct+1
                    ksc = rcpool.tile([P, 1], F32, tag="ksc", name="ksc")
                    nc.vector.tensor_copy(
                        ksc[0:HD, :], kv_sb[0:HD, base + HD:base + HD + 1])
                    nc.vector.tensor_copy(
                        ksc[HD:P, :],
                        kv_sb[HD:P, base + HE + HD:base + HE + HD + 1])
                    # kv[d,e] += ksum[d] * bv[e]  (junk halves harmless)
                    kvv = kv_sb[:, base:base + 2 * HE].rearrange(
                        "p (h e) -> p h e", e=HE)[:, :, :HD]
                    bvv = bv_rep[:, ct * P:(ct + 1) * P].rearrange(
                        "p (h e) -> p h e", e=HD)
                    nc.vector.scalar_tensor_tensor(
                        kvv, bvv, ksc[:], kvv, Alu.mult, Alu.add)
                    kvb = kvblocks[ct]
                    nc.vector.tensor_copy(
                        kvb[0:HD, 0:HD], kv_sb[0:HD, base:base + HD])
                    nc.gpsimd.tensor_copy(
                        kvb[HD:P, HD:2 * HD],
                        kv_sb[HD:P, base + HE:base + HE + HD])
                    nc.vector.tensor_copy(
                        kvb[0:HD, 2 * HD:2 * HD + 1],
                        kv_sb[0:HD, base + HD:base + HD + 1])
                    nc.gpsimd.tensor_copy(
                        kvb[HD:P, 2 * HD + 1:KB],
                        kv_sb[HD:P, base + HE + HD:base + HE + HD + 1])

            def b_chunk(cj):
                """Pass B for chunk cj: q^T projection + feature map."""
                xts = xt_t[cj]
                qft = []
                for ct in range(NCT):
                    ps = pps.tile([P, CHUNK], F32, tag="pps", name="qps")
                    for kt in range(NKT):
                        nc.tensor.matmul(
                            ps[:],
                            w_r["q"][:, kt * CG + ct * P: kt * CG + (ct + 1) * P],
                            xts[kt][:],
                            start=(kt == 0), stop=(kt == NKT - 1))
                    bcol = bq_sb[:, ct:ct + 1]
                    r = tmp.tile([P, CHUNK], BF16, tag="tr", name="t_qr")
                    nc.scalar.activation(r[:], ps[:], Act.Relu, bias=bcol)
                    qf = qftpool.tile([P, CHUNK], BF16, tag="qft", name="qft")
                    if ct % 2 == 0:
                        # min(exp(qb),1) + relu(qb) with the min fused into
                        # the DVE combine
                        e = tmp.tile([P, CHUNK], BF16, tag="te", name="t_qe")
                        nc.scalar.activation(e[:], ps[:], Act.Exp, bias=bcol)
                        nc.vector.scalar_tensor_tensor(
                            qf[:], e[:], 1.0, r[:], Alu.min, Alu.add)
                    else:
                        # same math with the min on ACT instead (keeps DVE
                        # under the PE): exp(-relu(-qb)) = min(exp(qb),1)
                        rn = tmp.tile([P, CHUNK], BF16, tag="tn", name="t_qn")
                        nc.scalar.activation(rn[:], ps[:], Act.Relu,
                                             bias=nbq_sb[:, ct:ct + 1],
                                             scale=-1.0)
                        e = tmp.tile([P, CHUNK], BF16, tag="te", name="t_qe")
                        nc.scalar.activation(e[:], rn[:], Act.Exp, scale=-1.0)
                        nc.vector.tensor_tensor(qf[:], e[:], r[:], Alu.add)
                    qft.append(qf)
                qft_c[cj] = qft

            def b_num(cj):
                """num/den matmuls + divide + store for chunk cj."""
                tok0 = cj * CHUNK
                qft = qft_c.pop(cj)
                for sub in range(NSUB):
                    osb = outpool.tile([P, CG], BF16, tag="out", name="osb")
                    for half in range(2):
                        # two head-pairs' [num 2x64 | den 2] per PSUM tile
                        pn = sps.tile([P, 2 * KB], F32, tag="sps", name="pn")
                        for c2 in range(2):
                            ct = half * 2 + c2
                            nc.tensor.matmul(
                                pn[:, c2 * KB:(c2 + 1) * KB],
                                qft[ct][:, sub * P:(sub + 1) * P],
                                kvblocks[ct][:],
                                start=True, stop=True)
                        rc = rcpool.tile([P, 4], F32, tag="rc", name="rc")
                        nc.vector.reciprocal(
                            rc[:].rearrange("p (c d) -> p c d", d=2),
                            pn[:].rearrange(
                                "p (c x) -> p c x", x=KB)[:, :, 2 * HD:KB])
                        # out = num * (1/den), recip broadcast over 64 cols
                        for c2 in range(2):
                            ct = half * 2 + c2
                            nc.vector.tensor_tensor(
                                osb[:, ct * P:(ct + 1) * P].rearrange(
                                    "p (h e) -> p h e", e=HD),
                                pn[:, c2 * KB:c2 * KB + 2 * HD].rearrange(
                                    "p (h e) -> p h e", e=HD),
                                rc[:, c2 * 2:(c2 + 1) * 2].rearrange(
                                    "p (h o) -> p h o", o=1
                                ).broadcast_to((P, 2, HD)),
                                Alu.mult)
                    nc.sync.dma_start(
                        out_d[tok0 + sub * P: tok0 + (sub + 1) * P, :],
                        osb[:])

            # ---- software-pipelined stream ----
            for ci in range(NCHUNK):
                a_chunk(ci)
                if ci >= 1:
                    a_kv(ci - 1)
            b_chunk(0)          # q^T needs no kv; bridges the A->B gap
            for ct in range(NCT):
                nc.gpsimd.memset(kvblocks[ct][:], 0.0)
            a_kv(NCHUNK - 1)
            build_kvblocks()
            for cj in range(1, NCHUNK):
                b_chunk(cj)
                b_num(cj - 1)
            b_num(NCHUNK - 1)

    nc.compile()
    return nc


def _get_nc():
    global _CACHED_NC
    if _CACHED_NC is None:
        _CACHED_NC = _build()
    return _CACHED_NC


def _make_in_maps(hidden_states, Wq, bq, Wk, bk, Wv, bv):
    import ml_dtypes

    bf16 = ml_dtypes.bfloat16
    hs = np.asarray(hidden_states, np.float32)
    arrs = {"wq": np.asarray(Wq, np.float32), "wk": np.asarray(Wk, np.float32),
            "wv": np.asarray(Wv, np.float32), "bq": np.asarray(bq, np.float32),
            "bk": np.asarray(bk, np.float32), "bv": np.asarray(bv, np.float32)}
    xts = [np.ascontiguousarray(hs[b].T).astype(bf16) for b in range(B)]
    in_maps = []
    for c in range(NCORES):
        b, g = divmod(c, 2)
        sl = slice(g * CG, (g + 1) * CG)
        in_maps.append({
            "xt": xts[b],
            "wq": np.ascontiguousarray(arrs["wq"][:, sl]).astype(bf16),
            "wk": np.ascontiguousarray(arrs["wk"][:, sl]).astype(bf16),
            "wv": np.ascontiguousarray(arrs["wv"][:, sl]).astype(bf16),
            "bq": np.ascontiguousarray(arrs["bq"][sl]),
            "bk": np.ascontiguousarray(arrs["bk"][sl]).reshape(1, CG),
            "bv": np.ascontiguousarray(arrs["bv"][sl]).reshape(1, CG),
        })
    return in_maps


def _run(in_maps, **kwargs):
    from concourse.bass_utils import run_bass_kernel_spmd
    nc = _get_nc()
    return run_bass_kernel_spmd(nc, in_maps, core_ids=list(range(NCORES)),
                                **kwargs)


def _assemble(results):
    out = np.empty((B, S, D), np.float32)
    for c in range(NCORES):
        b, g = divmod(c, 2)
        out[b, :, g * CG:(g + 1) * CG] = np.asarray(
            results[c]["out"], np.float32)
    return out


def kernel(hidden_states, Wq, bq, Wk, bk, Wv, bv):
    in_maps = _make_in_maps(hidden_states, Wq, bq, Wk, bk, Wv, bv)
    res = _run(in_maps)
    return _assemble(res.results)
